# revision 1
# baseline (speedup 1.0000x reference)
"""Trainium2 Bass kernel for nn_Decoder (attention + LSTM decoder).

Contract: kernel(**inputs) takes FULL unsharded inputs (as in
reference.setup_inputs()) and returns the FULL [256, 1] float32 output.

Strategy: data-parallel over batch B=256 across 8 NeuronCores (32 batch
rows per core); small weights replicated. The T-1=127 step recurrence is
sequential; per-core per-step work is laid out to keep the 128-lane
engines full:

  - encpT/hsum/hT in [E=128 partitions, (b,t) free] fp16, t padded to 128
  - per-step broadcast-add of the attention state projection via 32
    tensor_scalar adds (per-partition scalar = column of attn proj)
  - tanh on ScalarE in chunks (last chunks smaller so the trailing masked
    score matmuls expose less latency)
  - scores via 32 masked matmuls (W2 in column b of block b) accumulating
    into one [32, 127] PSUM tile
  - softmax without max-subtraction (|scores| <= ||W2||_1 ~ 9, exp safe),
    exp+sum fused via activation(accum_out=...)
  - y_tilde via precomputed XWf = X @ Wf[:E]  (context never materialized
    per step; only after the final step for the output head)
  - LSTM states stored doubled (D=2d, C=2c) in [128, 32] T-layout; all
    sigmoids via tanh identities so a single ACT table set is used;
    0.5 factors and the doubled g-gate are folded into host-prepped weights

Scaled-weight algebra (validated in numpy): with states D=2d, C=2c,
  gates_pre = (0.5*s_g*W_hh_g)^T D + s_g*W_ih_g*y + s_g*b_g, s_g = 2 for
  the g gate else 1; tanh(0.5*gates_pre) gives tanh(x/2) for i,f,o and
  tanh(x_g) for g. Then sigma(x) = 0.5*(1+tanh(x/2)) and
  C' = 0.5*(tf+1)*C + (ti+1)*tg, tanh(c') = tanh(0.5*C'),
  D' = (to+1)*tanh(c').
"""
import sys

sys.path.insert(0, "/opt/trn_rl_repo")

import numpy as np

import concourse.bass as bass
import concourse.mybir as mybir
import concourse.tile as tile

B, TM1, E, D = 256, 127, 128, 128
NCORES = 8
Bc = B // NCORES  # 32
TP = 128  # padded t stride
F16 = mybir.dt.float16
F32 = mybir.dt.float32
AF = mybir.ActivationFunctionType
OP = mybir.AluOpType

# tanh chunking in b-blocks; smaller tail chunks shorten the exposed
# post-tanh score-matmul chain
CHUNKS = (4, 8, 8, 8, 4)
assert sum(CHUNKS) == Bc


def _split_ctrl_waits(nc, max_waits=1):
    """walrus in this env rejects instructions with more than one sem wait
    ("Too many sync wait commands", e.g. Matmult/S3_LW and Drain/CTRL
    structs). Hoist excess waits onto dedicated NOPs on the same engine,
    which execute in queue order before the original instruction —
    identical blocking semantics."""
    for fn in nc.m.functions:
        for bb in fn.blocks:
            new_insts = []
            for ins in bb.instructions:
                si = getattr(ins, "sync_info", None)
                if si is not None and si.on_wait and len(si.on_wait) > max_waits:
                    waits = list(si.on_wait)
                    keep = waits[-max_waits:]
                    for k, w in enumerate(waits[:-max_waits]):
                        new_insts.append(
                            mybir.InstNoOp(
                                name=f"{ins.name}-wsplit{k}",
                                engine=ins.engine,
                                sync_info=mybir.SyncInfo(on_wait=[w], on_update=[]),
                                bass_nofuse=True,
                            )
                        )
                    si.on_wait = keep
                new_insts.append(ins)
            bb.instructions = new_insts
    return nc


def build_kernel(steps=TM1, fix_waits=True):
    """Emit the per-core Bass/Tile kernel. Same NEFF runs SPMD on all 8
    cores; only the DRAM input contents differ per core."""
    nc = bass.Bass()

    # per-core tensors
    xt_d = nc.dram_tensor("xt", [E, Bc * TP], F16, kind="ExternalInput")
    xte_d = nc.dram_tensor("xte", [TM1, Bc * E], F32, kind="ExternalInput")
    yfix_d = nc.dram_tensor("yfix", [Bc, TM1], F32, kind="ExternalInput")
    # replicated weights (host-prepped)
    w1e_d = nc.dram_tensor("w1e", [E, E], F16, kind="ExternalInput")
    b1r_d = nc.dram_tensor("b1r", [1, E], F16, kind="ExternalInput")
    ones_d = nc.dram_tensor("onesr", [1, Bc * TP], F16, kind="ExternalInput")
    w1ds_d = nc.dram_tensor("w1ds", [D, E], F32, kind="ExternalInput")
    w1cs_d = nc.dram_tensor("w1cs", [D, E], F32, kind="ExternalInput")
    whh_d = nc.dram_tensor("whh", [D, 4 * D], F32, kind="ExternalInput")
    wihb_d = nc.dram_tensor("wihb", [2, 4 * D], F32, kind="ExternalInput")
    w2m_d = nc.dram_tensor("w2m", [E, Bc * Bc], F16, kind="ExternalInput")
    wfm_d = nc.dram_tensor("wfm", [E, Bc * Bc], F16, kind="ExternalInput")
    wffd_d = nc.dram_tensor("wffd", [D, 1], F32, kind="ExternalInput")
    wffc_d = nc.dram_tensor("wffc", [E, 1], F32, kind="ExternalInput")
    bffr_d = nc.dram_tensor("bffr", [1, 1], F32, kind="ExternalInput")
    out_d = nc.dram_tensor("yout", [1, Bc], F32, kind="ExternalOutput")

    with tile.TileContext(nc) as tc:
        with (
            tc.tile_pool(name="const", bufs=1) as cpool,
            tc.tile_pool(name="work", bufs=1) as wpool,
            tc.tile_pool(name="state", bufs=1) as spool,
            tc.tile_pool(name="psum2", bufs=2, space="PSUM") as ppool2,
        ):
            # ---- load constants / inputs ----
            xt = cpool.tile([E, Bc * TP], F16)
            xte = cpool.tile([TM1, Bc * E], F32)
            yfix = cpool.tile([Bc, TM1], F32)
            w1e = cpool.tile([E, E], F16)
            b1r = cpool.tile([1, E], F16)
            onesr = cpool.tile([1, Bc * TP], F16)
            w1ds = cpool.tile([D, E], F32)
            w1cs = cpool.tile([D, E], F32)
            whh = cpool.tile([D, 4 * D], F32)
            wihb = cpool.tile([2, 4 * D], F32)
            w2m = cpool.tile([E, Bc * Bc], F16)
            wfm = cpool.tile([E, Bc * Bc], F16)
            wffd = cpool.tile([D, 1], F32)
            wffc = cpool.tile([E, 1], F32)
            bffr = cpool.tile([1, 1], F32)
            for sb, dr in [
                (xt, xt_d), (xte, xte_d), (yfix, yfix_d), (w1e, w1e_d),
                (b1r, b1r_d), (onesr, ones_d), (w1ds, w1ds_d), (w1cs, w1cs_d),
                (whh, whh_d), (wihb, wihb_d), (w2m, w2m_d), (wfm, wfm_d),
                (wffd, wffd_d), (wffc, wffc_d), (bffr, bffr_d),
            ]:
                nc.sync.dma_start(sb[:], dr[:])

            # ---- persistent SBUF buffers ----
            encp = cpool.tile([E, Bc * TP], F16)
            xwf = cpool.tile([Bc, TM1], F32)
            hsum = [cpool.tile([E, nb * TP], F16, name=f"hsum{k}")
                    for k, nb in enumerate(CHUNKS)]
            ht = [cpool.tile([E, nb * TP], F16, name=f"ht{k}")
                  for k, nb in enumerate(CHUNKS)]
            # LSTM state ping-pong (D=2d, C=2c), zero-initialized
            dt_s = [spool.tile([D, Bc], F32, name=f"dt{i}") for i in range(2)]
            ct_s = [spool.tile([D, Bc], F32, name=f"ct{i}") for i in range(2)]
            nc.vector.memset(dt_s[0][:], 0.0)
            nc.vector.memset(ct_s[0][:], 0.0)
            nc.vector.memset(dt_s[1][:], 0.0)
            nc.vector.memset(ct_s[1][:], 0.0)
            # y_tilde staging: col0 = y_tilde, col1 = ones
            ytld = spool.tile([Bc, 32], F32)
            nc.vector.memset(ytld[:], 0.0)
            nc.vector.memset(ytld[:, 1:2], 1.0)
            ytldT = spool.tile([Bc, 32], F32)

            # ---- init phase: enc_proj and XWf ----
            NCE = 8  # enc-proj chunks of 512 cols
            ew = (Bc * TP) // NCE
            for k in range(NCE):
                encps = ppool2.tile([E, ew], F32, name="encps", tag="pscratch")
                sl = slice(k * ew, (k + 1) * ew)
                nc.tensor.matmul(encps[:], w1e[:], xt[:, sl], start=True, stop=False)
                nc.tensor.matmul(encps[:], b1r[:], onesr[:, sl], start=False, stop=True)
                nc.vector.tensor_copy(encp[:, sl], encps[:])

            xwfp = ppool2.tile([Bc, TM1], F32, name="xwfp", tag="pscratch")
            for b in range(Bc):
                nc.tensor.matmul(
                    xwfp[:],
                    wfm[:, b * Bc:(b + 1) * Bc],
                    xt[:, b * TP:b * TP + TM1],
                    start=(b == 0),
                    stop=(b == Bc - 1),
                )
            nc.vector.tensor_copy(xwf[:], xwfp[:])

            # ---- recurrence ----
            exp_s = None
            rinv = None
            for t in range(steps):
                DT = dt_s[t % 2]
                CT = ct_s[t % 2]
                DTn = dt_s[(t + 1) % 2]
                CTn = ct_s[(t + 1) % 2]

                # attention state projection -> [E, Bc] psum -> sbuf
                # (C half was issued at the end of the previous step, right
                # after CT became available, overlapping tanh_c)
                if t == 0:
                    attp = ppool2.tile([E, Bc], F32, name="attp")
                    nc.tensor.matmul(attp[:], w1cs[:], CT[:], start=True, stop=False)
                nc.tensor.matmul(attp[:], w1ds[:], DT[:], start=False, stop=True)
                atts = wpool.tile([E, Bc], F32, name="atts")
                nc.vector.tensor_copy(atts[:], attp[:])

                # gates psum: W_hh part (kick early; W_ih/bias part later)
                gps = ppool2.tile([D, 4 * Bc], F32, name="gps")
                for g in range(4):
                    nc.tensor.matmul(
                        gps[:, g * Bc:(g + 1) * Bc],
                        whh[:, g * D:(g + 1) * D],
                        DT[:],
                        start=(g == 0),
                        stop=False,
                    )

                # scores psum; broadcast-add + tanh + masked MMs, chunked
                scp = ppool2.tile([Bc, TM1], F32, name="scp")
                b = 0
                for k, nb in enumerate(CHUNKS):
                    for j in range(nb):
                        nc.vector.tensor_scalar_add(
                            hsum[k][:, j * TP:(j + 1) * TP],
                            encp[:, (b + j) * TP:(b + j + 1) * TP],
                            atts[:, b + j:b + j + 1],
                        )
                    nc.scalar.activation(ht[k][:], hsum[k][:], AF.Tanh)
                    for j in range(nb):
                        bb = b + j
                        nc.tensor.matmul(
                            scp[:],
                            w2m[:, bb * Bc:(bb + 1) * Bc],
                            ht[k][:, j * TP:j * TP + TM1],
                            start=(bb == 0),
                            stop=(bb == Bc - 1),
                        )
                    b += nb

                # softmax pieces (no max subtraction; |scores| <= ~9)
                exp_s = wpool.tile([Bc, TM1], F32, name="exps")
                sume = wpool.tile([Bc, 1], F32, name="sume")
                nc.scalar.activation(exp_s[:], scp[:], AF.Exp, accum_out=sume[:])
                rinv = wpool.tile([Bc, 1], F32, name="rinv")
                nc.vector.reciprocal(rinv[:], sume[:])

                # y_tilde = (sum_t exp*XWf) * rinv + yfix[:, t]
                ttr_o = wpool.tile([Bc, TM1], F32, name="ttro")
                ydot = wpool.tile([Bc, 1], F32, name="ydot")
                nc.vector.scalar_tensor_tensor(
                    ttr_o[:], exp_s[:], 1.0, xwf[:], OP.mult, OP.mult,
                    accum_out=ydot[:],
                )
                nc.vector.scalar_tensor_tensor(
                    ytld[:, 0:1], ydot[:], rinv[:, 0:1], yfix[:, t:t + 1],
                    OP.mult, OP.add,
                )
                nc.vector.transpose(ytldT[:], ytld[:])

                # gates: W_ih/bias part
                for g in range(4):
                    nc.tensor.matmul(
                        gps[:, g * Bc:(g + 1) * Bc],
                        wihb[:, g * D:(g + 1) * D],
                        ytldT[0:2, :],
                        start=False,
                        stop=(g == 3),
                    )

                # LSTM cell
                tg = wpool.tile([D, 4 * Bc], F32, name="tg")
                nc.scalar.activation(tg[:], gps[:], AF.Tanh, scale=0.5)
                a_sb = wpool.tile([D, Bc], F32, name="asb")
                b_sb = wpool.tile([D, Bc], F32, name="bsb")
                nc.vector.scalar_tensor_tensor(
                    a_sb[:], tg[:, Bc:2 * Bc], 1.0, CT[:], OP.add, OP.mult)
                nc.vector.scalar_tensor_tensor(
                    b_sb[:], tg[:, 0:Bc], 1.0, tg[:, 2 * Bc:3 * Bc], OP.add, OP.mult)
                nc.vector.scalar_tensor_tensor(
                    CTn[:], a_sb[:], 0.5, b_sb[:], OP.mult, OP.add)
                tc_sb = wpool.tile([D, Bc], F32, name="tcsb")
                nc.scalar.activation(tc_sb[:], CTn[:], AF.Tanh, scale=0.5)
                # next step's attn C-half overlaps tanh_c / DTn
                if t < steps - 1:
                    attp = ppool2.tile([E, Bc], F32, name="attp")
                    nc.tensor.matmul(attp[:], w1cs[:], CTn[:], start=True, stop=False)
                nc.vector.scalar_tensor_tensor(
                    DTn[:], tg[:, 3 * Bc:4 * Bc], 1.0, tc_sb[:], OP.add, OP.mult)

            # ---- final: context + output head ----
            DT = dt_s[steps % 2]
            beta = wpool.tile([Bc, TP], F32, name="beta")
            nc.vector.memset(beta[:], 0.0)
            nc.vector.tensor_scalar_mul(beta[:, 0:TM1], exp_s[:], rinv[:, 0:1])
            betaT = wpool.tile([TP, Bc], F32, name="betaT")
            for blk in range(4):
                nc.vector.transpose(
                    betaT[blk * 32:(blk + 1) * 32, :],
                    beta[:, blk * 32:(blk + 1) * 32],
                )
            bmask = wpool.tile([TM1, Bc * Bc], F32, name="bmask")
            nc.vector.memset(bmask[:], 0.0)
            nc.vector.tensor_copy(bmask[:, 0:Bc * Bc:Bc + 1], betaT[0:TM1, :])
            ctxp = ppool2.tile([E, Bc], F32, name="ctxp", tag="pscratch")
            for b in range(Bc):
                nc.tensor.matmul(
                    ctxp[:],
                    xte[:, b * E:(b + 1) * E],
                    bmask[:, b * Bc:(b + 1) * Bc],
                    start=(b == 0),
                    stop=(b == Bc - 1),
                )
            ctxs = wpool.tile([E, Bc], F32, name="ctxs")
            nc.vector.tensor_copy(ctxs[:], ctxp[:])
            # y = wffd^T D + wffc^T ctx + bff
            yp = ppool2.tile([1, Bc], F32, name="yp", tag="pscratch")
            nc.tensor.matmul(yp[:], wffd[:], DT[:], start=True, stop=False)
            nc.tensor.matmul(yp[:], wffc[:], ctxs[:], start=False, stop=True)
            ysb = wpool.tile([1, Bc], F32, name="ysb")
            # + bff folded into the PSUM->SBUF move
            nc.vector.tensor_scalar_add(ysb[:], yp[:], bffr[0:1, 0:1])
            nc.sync.dma_start(out_d[:], ysb[:])

    if fix_waits:
        _split_ctrl_waits(nc)
    return nc


def prep_inputs(inputs):
    """Host-side sharding + weight prep. Returns list of 8 in_maps."""
    f16 = np.float16
    X = np.asarray(inputs["X_encoded"], np.float32)
    y_prev = np.asarray(inputs["y_prev"], np.float32)
    W1 = np.asarray(inputs["W1"], np.float32)
    b1 = np.asarray(inputs["b1"], np.float32)
    W2 = np.asarray(inputs["W2"], np.float32)
    W_ih = np.asarray(inputs["W_ih"], np.float32)
    W_hh = np.asarray(inputs["W_hh"], np.float32)
    b_ih = np.asarray(inputs["b_ih"], np.float32)
    b_hh = np.asarray(inputs["b_hh"], np.float32)
    Wf = np.asarray(inputs["Wf"], np.float32)
    bf = np.asarray(inputs["bf"], np.float32)
    Wff = np.asarray(inputs["Wff"], np.float32)
    bff = np.asarray(inputs["bff"], np.float32)

    W1_d, W1_c, W1_e = W1[:D], W1[D:2 * D], W1[2 * D:]
    gsc = np.array([1.0, 1.0, 2.0, 1.0], np.float32)

    w1ds = np.ascontiguousarray(0.5 * W1_d)
    w1cs = np.ascontiguousarray(0.5 * W1_c)
    whh = np.zeros((D, 4 * D), np.float32)
    wihb = np.zeros((2, 4 * D), np.float32)
    for g in range(4):
        whh[:, g * D:(g + 1) * D] = (0.5 * gsc[g] * W_hh[g * D:(g + 1) * D, :]).T
        wihb[0, g * D:(g + 1) * D] = gsc[g] * W_ih[g * D:(g + 1) * D, 0]
        wihb[1, g * D:(g + 1) * D] = gsc[g] * (b_ih + b_hh)[g * D:(g + 1) * D]
    w2m = np.zeros((E, Bc * Bc), f16)
    wfm = np.zeros((E, Bc * Bc), f16)
    for b in range(Bc):
        w2m[:, b * Bc + b] = W2[:, 0].astype(f16)
        wfm[:, b * Bc + b] = Wf[:E, 0].astype(f16)
    shared = {
        "w1e": W1_e.astype(f16),
        "b1r": b1.reshape(1, E).astype(f16),
        "onesr": np.ones((1, Bc * TP), f16),
        "w1ds": w1ds, "w1cs": w1cs, "whh": whh, "wihb": wihb,
        "w2m": w2m, "wfm": wfm,
        "wffd": np.ascontiguousarray(0.5 * Wff[:D, 0:1]),
        "wffc": np.ascontiguousarray(Wff[D:, 0:1]),
        "bffr": np.array([[bff[0]]], np.float32),
    }
    in_maps = []
    for c in range(NCORES):
        sl = slice(c * Bc, (c + 1) * Bc)
        Xc = X[sl]
        xtp = np.zeros((E, Bc * TP), f16)
        xtp3 = xtp.reshape(E, Bc, TP)
        xtp3[:, :, :TM1] = Xc.transpose(2, 0, 1).astype(f16)
        xte = np.ascontiguousarray(
            Xc.transpose(1, 0, 2).reshape(TM1, Bc * E).astype(np.float32))
        yfix = (y_prev[sl] * Wf[E, 0] + bf[0]).astype(np.float32)
        in_maps.append({
            "xt": np.ascontiguousarray(xtp),
            "xte": xte,
            "yfix": np.ascontiguousarray(yfix),
            **shared,
        })
    return in_maps


_CACHED = {}


def run(inputs, trace=False, **kw):
    from concourse.bass_utils import run_bass_kernel_spmd

    if "nc" not in _CACHED:
        _CACHED["nc"] = build_kernel()
    nc = _CACHED["nc"]
    in_maps = prep_inputs(inputs)
    res = run_bass_kernel_spmd(
        nc, in_maps, core_ids=list(range(NCORES)), trace=trace, **kw
    )
    out = np.zeros((B, 1), np.float32)
    for c in range(NCORES):
        out[c * Bc:(c + 1) * Bc, 0] = res.results[c]["yout"][0]
    return out, res


def kernel(**inputs) -> np.ndarray:
    return run(inputs)[0]

